# revision 1
# baseline (speedup 1.0000x reference)
"""Chamfer distance (symmetric, weighted forward) on 8 Trainium2 NeuronCores.

Strategy
--------
Brute-force all-pairs squared distances on the TensorEngine via the augmented
matmul  ||s||^2 + ||t||^2 - 2 s.t  with every fp32 operand split into 3 bf16
planes (products of bf16 planes are exact in fp32), so the PE computes
fp32-accurate squared distances at full bf16 streaming speed.

Sharding: 2 cores per batch element (B=4), each core takes 4096 of the 8192
source rows x all 8192 targets:
  - per-source min over targets (forward): fused DVE tensor_tensor_reduce
    (pairwise-min of tile halves + free-axis min-reduce, chained via the
    scalar initial value).
  - per-target min: elementwise min fold across 32 source blocks into
    [128, 8192] accumulators; the final 128-way cross-partition min plus the
    cross-core min happen on the host.

The host builds the bf16 split planes, runs the SPMD kernel, and computes the
final scalar. Device minima below SMALL_SQ_THRESH are re-evaluated in fp64 on
the host: sqrt amplifies the PE's ~4e-7 absolute fp32-accumulation noise for
near-coincident pairs, so those few values come from an exact recompute.
"""

import os
import sys

import numpy as np

for _p in ("/root/.axon_site", "/root/.axon_site/_ro/trn_rl_repo", "/root/.axon_site/_ro/pypackages"):
    if os.path.isdir(_p) and _p not in sys.path:
        sys.path.append(_p)

import ml_dtypes

BF16 = ml_dtypes.bfloat16

# Problem constants (hardcoded per spec)
B = 4
N = 8192  # sources per batch
M = 8192  # targets per batch
NCORES = 8
SRC_PER_CORE = N // 2        # 4096
NBLK = SRC_PER_CORE // 128   # 32 source blocks per core
NSUP = M // 2048             # 4 target supertiles per batch
KROWS = 32
EPS = 1e-8
SMALL_SQ_THRESH = 1e-4

_PROGRAM = None  # cached (nc, ...) build


def _splitn(x, n):
    """Split fp64 array into n bf16 planes summing (to ~8n bits) to x."""
    x = x.astype(np.float64)
    out = []
    for _ in range(n):
        a = x.astype(BF16)
        out.append(a)
        x = x - a.astype(np.float64)
    return out


def _build_planes(src_b, tgt_b):
    """Augmented K=32 bf16 planes for one batch.

    Returns L [32, N] (source side / lhsT) and R [32, M] (target side / rhs)
    such that sum_k L[k, n] * R[k, m] == ||s_n - t_m||^2 up to fp32 rounding.
    """
    sa, sb, sc = _splitn(-2.0 * src_b.astype(np.float64), 3)  # (N, 3) each
    ta, tb, tc = _splitn(tgt_b.astype(np.float64), 3)
    ns = (src_b.astype(np.float64) ** 2).sum(1)
    nt = (tgt_b.astype(np.float64) ** 2).sum(1)
    nss = _splitn(ns, 4)
    nts = _splitn(nt, 4)
    one_s = np.ones(ns.shape, BF16)
    one_t = np.ones(nt.shape, BF16)
    Ls, Rs = [], []
    for k in range(3):
        # products: ad ae af bd be bf cd ce (only c*f dropped, ~2^-32 rel)
        for (u, v) in [(sa, ta), (sa, tb), (sa, tc), (sb, ta), (sb, tb), (sb, tc), (sc, ta), (sc, tb)]:
            Ls.append(u[:, k])
            Rs.append(v[:, k])
    for u in nss:
        Ls.append(u)
        Rs.append(one_t)
    for v in nts:
        Ls.append(one_s)
        Rs.append(v)
    L = np.ascontiguousarray(np.stack(Ls, 0).astype(BF16))
    R = np.ascontiguousarray(np.stack(Rs, 0).astype(BF16))
    return L, R


def _build_program():
    """Build the SPMD Tile program once. Returns the finalized Bass object."""
    import concourse.bacc as bacc
    import concourse.tile as tile
    from concourse import mybir

    nc = bacc.Bacc("TRN2", target_bir_lowering=False, debug=False, num_devices=NCORES)

    lhsT_d = nc.dram_tensor("lhsT", [KROWS, SRC_PER_CORE], mybir.dt.bfloat16, kind="ExternalInput")
    rhs_d = nc.dram_tensor("rhs", [KROWS, M], mybir.dt.bfloat16, kind="ExternalInput")
    s2t_d = nc.dram_tensor("s2t", [SRC_PER_CORE, 1], mybir.dt.float32, kind="ExternalOutput")
    t2s_d = nc.dram_tensor("t2s", [128, M], mybir.dt.float32, kind="ExternalOutput")

    FMIN = mybir.AluOpType.min
    FMAX = mybir.AluOpType.max

    with tile.TileContext(nc) as tc:
        with (
            tc.tile_pool(name="weights", bufs=1) as wpool,
            tc.tile_pool(name="psum", bufs=2, space="PSUM") as pspool,
            tc.tile_pool(name="accs", bufs=4) as apool,
            tc.tile_pool(name="evac", bufs=4) as epool,
            tc.tile_pool(name="folds", bufs=1) as fpool,
        ):
            lhsT_sb = wpool.tile([KROWS, SRC_PER_CORE], mybir.dt.bfloat16)
            rhs_sb = wpool.tile([KROWS, M], mybir.dt.bfloat16)
            nc.sync.dma_start(out=lhsT_sb, in_=lhsT_d[:, :])
            nc.sync.dma_start(out=rhs_sb, in_=rhs_d[:, :])

            A = [
                fpool.tile([128, 2048], mybir.dt.float32, name=f"A{c}", tag=f"A{c}")
                for c in range(NSUP)
            ]

            for b in range(NBLK):
                wslice = lhsT_sb[:, b * 128:(b + 1) * 128]
                racc = apool.tile([128, NSUP], mybir.dt.float32)
                acc = apool.tile([128, 1], mybir.dt.float32)
                for c in range(NSUP):
                    ps = pspool.tile([128, 2048], mybir.dt.float32)
                    for q in range(4):
                        j = c * 4 + q
                        nc.tensor.matmul(
                            ps[:, q * 512:(q + 1) * 512],
                            wslice,
                            rhs_sb[:, j * 512:(j + 1) * 512],
                            start=True,
                            stop=True,
                        )
                    # ACT evacuates PSUM->SBUF negated (scale=-1), so both
                    # min-reductions become max ops; pool_max is single-source
                    # and can exceed tensor_reduce's 1x rate from SBUF.
                    ev = epool.tile([128, 2048], mybir.dt.float32)
                    nc.scalar.activation(ev, ps, mybir.ActivationFunctionType.Copy, scale=-1.0)
                    nc.vector.pool_max(racc[:, c:c + 1], ev)
                    if b == 0:
                        nc.vector.tensor_copy(A[c], ev)
                    else:
                        nc.vector.tensor_tensor(A[c], A[c], ev, FMAX)
                nc.vector.tensor_reduce(acc, racc, axis=mybir.AxisListType.X, op=FMAX)
                nc.sync.dma_start(out=s2t_d[b * 128:(b + 1) * 128, :], in_=acc)

            for c in range(NSUP):
                nc.sync.dma_start(out=t2s_d[:, c * 2048:(c + 1) * 2048], in_=A[c])

    nc.compile()
    return nc


def _get_program():
    global _PROGRAM
    if _PROGRAM is None:
        _PROGRAM = _build_program()
    return _PROGRAM


def _exact_minsq_fp64(pts, others):
    """Exact (fp64) min squared distance from each of pts to the set others."""
    p = pts.astype(np.float64)
    o = others.astype(np.float64)
    no = (o * o).sum(1)
    out = np.empty(len(p), np.float64)
    for i0 in range(0, len(p), 2048):
        pp = p[i0:i0 + 2048]
        sq = ((pp * pp).sum(1))[:, None] + no[None, :] - 2.0 * (pp @ o.T)
        out[i0:i0 + 2048] = sq.min(1)
    return np.maximum(out, 0.0)


def kernel(source, target, weights):
    from concourse.bass_utils import run_bass_kernel_spmd

    source = np.asarray(source)
    target = np.asarray(target)
    weights = np.asarray(weights)

    in_maps = []
    planes = [_build_planes(source[b], target[b]) for b in range(B)]
    for i in range(NCORES):
        b, half = i // 2, i % 2
        L, R = planes[b]
        in_maps.append({
            "lhsT": np.ascontiguousarray(L[:, half * SRC_PER_CORE:(half + 1) * SRC_PER_CORE]),
            "rhs": R,
        })

    nc = _get_program()
    res = None
    last_err = None
    for attempt in range(3):
        try:
            res = run_bass_kernel_spmd(nc, in_maps, list(range(NCORES))).results
            break
        except Exception as e:  # transient device wedge: retry
            last_err = e
            import time as _time

            _time.sleep(5.0 * (attempt + 1))
    if res is None:
        raise last_err

    s_minsq = np.empty((B, N), np.float64)
    t_minsq = np.empty((B, M), np.float64)
    for b in range(B):
        lo = -res[2 * b]["s2t"].reshape(-1).astype(np.float64)
        hi = -res[2 * b + 1]["s2t"].reshape(-1).astype(np.float64)
        s_minsq[b] = np.maximum(np.concatenate([lo, hi]), 0.0)
        fold = np.maximum(res[2 * b]["t2s"], res[2 * b + 1]["t2s"])
        t_minsq[b] = np.maximum(-fold.max(0), 0.0)

    # Host fp64 re-evaluation where sqrt amplifies device fp32 noise.
    for b in range(B):
        bad = np.flatnonzero(s_minsq[b] < SMALL_SQ_THRESH)
        if len(bad):
            s_minsq[b, bad] = _exact_minsq_fp64(source[b, bad], target[b])
        bad = np.flatnonzero(t_minsq[b] < SMALL_SQ_THRESH)
        if len(bad):
            t_minsq[b, bad] = _exact_minsq_fp64(target[b, bad], source[b])

    fwd = float((np.sqrt(s_minsq + EPS) * weights.astype(np.float64)).mean())
    bwd = float(np.sqrt(t_minsq + EPS).mean())
    return np.float32(fwd + bwd)



# revision 12
# speedup vs baseline: 8.6757x; 8.6757x over previous
"""Chamfer distance (symmetric, weighted forward) on 8 Trainium2 NeuronCores.

Strategy: grid-pruned nearest-neighbor search ("cell lists").
----------------------------------------------------------------
Both point sets of a batch are binned into an 18^3 rectilinear grid whose
boundaries are N(0,1) quantiles (coords are iid normal => near-uniform cell
occupancy). Points are ordered by the Hilbert index of their cell; each block
of 128 rows scans only the targets in the 1-ring of the block's cells
(host-gathered, padded to a static W=512 window). Each of the 8 cores handles
one (batch, direction) job: 64 blocks x [K=32] x [128 rows x 512 candidates]
augmented matmuls (negated, so PSUM holds -||s-t||^2; fp32 operands split into
bf16 planes, products exact in fp32). PSUM is evacuated to bf16 SBUF (ScalarE
activation for most groups, DVE tensor_copy for the rest, keeping both engines
busy) and a grouped pool_max produces the per-row maxima of -sq = -min sq.

Exactness: a windowed min is provably exact when it is smaller than the
distance from the point to the boundary of its own cell's 1-ring (r_cover).
The host re-evaluates (fp64) the few rows failing that certificate, rows of
overflowing/empty blocks, and near-zero mins where sqrt amplifies fp noise.
This holds for ANY input data, not just the benchmarked distribution.
"""

import os
import sys

import numpy as np

for _p in ("/root/.axon_site", "/root/.axon_site/_ro/trn_rl_repo", "/root/.axon_site/_ro/pypackages"):
    if os.path.isdir(_p) and _p not in sys.path:
        sys.path.append(_p)

import ml_dtypes

BF16 = ml_dtypes.bfloat16

# Problem constants (hardcoded per spec)
B = 4
N = 8192          # sources per batch
M = 8192          # targets per batch
NCORES = 8
KROWS = 32        # bf16 planes of the augmented matmul
NBLK = N // 128   # 64 row blocks per job
W = 512           # candidate window per block
G = 18            # grid resolution per axis
HBITS = 5
EPS = 1e-8
SMALL_SQ_THRESH = 4e-4
CERT_MARGIN = 0.98

# Interior N(0,1) quantile boundaries of the G=18 grid (exact same grid the
# certificate radii are computed from).
QS_IN = np.array([
    -1.59321882, -1.22064035, -0.967421566, -0.764709674, -0.589455798,
    -0.430727299, -0.282216147, -0.139710299, 0.0, 0.139710299, 0.282216147,
    0.430727299, 0.589455798, 0.764709674, 0.967421566, 1.22064035, 1.59321882,
], np.float64)
QS = np.concatenate([[-np.inf], QS_IN, [np.inf]])  # length G+1

# Blocks are processed in groups of 4 sharing one pool_max reduction.
# Per-group evacuation path: True -> ScalarE activation, False -> DVE
# tensor_copy. The mix keeps both engines busy.
GROUP = 4
NGRP = NBLK // GROUP
ACT_GROUP = [g % 4 != 3 for g in range(NGRP)]

_PROGRAM = None  # cached compiled Bass program


def _splitn(x, n):
    """Split fp64 array into n bf16 planes summing (to ~8n bits) to x."""
    x = x.astype(np.float64)
    out = []
    for _ in range(n):
        a = x.astype(BF16)
        out.append(a)
        x = x - a.astype(np.float64)
    return out


def _build_planes(src_b, tgt_b):
    """Augmented K=32 bf16 planes: sum_k L[k,n] R[k,m] == ||s_n - t_m||^2."""
    sa, sb, sc = _splitn(-2.0 * src_b.astype(np.float64), 3)
    ta, tb, tc = _splitn(tgt_b.astype(np.float64), 3)
    ns = (src_b.astype(np.float64) ** 2).sum(1)
    nt = (tgt_b.astype(np.float64) ** 2).sum(1)
    nss = _splitn(ns, 4)
    nts = _splitn(nt, 4)
    one_s = np.ones(ns.shape, BF16)
    one_t = np.ones(nt.shape, BF16)
    Ls, Rs = [], []
    for k in range(3):
        for (u, v) in [(sa, ta), (sa, tb), (sa, tc), (sb, ta), (sb, tb), (sb, tc), (sc, ta), (sc, tb)]:
            Ls.append(u[:, k])
            Rs.append(v[:, k])
    for u in nss:
        Ls.append(u)
        Rs.append(one_t)
    for v in nts:
        Ls.append(one_s)
        Rs.append(v)
    L = np.ascontiguousarray(np.stack(Ls, 0).astype(BF16))
    R = np.ascontiguousarray(np.stack(Rs, 0).astype(BF16))
    return L, R


def _hilbert_key(c, bits=HBITS):
    """Hilbert index of integer 3d cells (Skilling transpose algorithm)."""
    X = c.astype(np.int64).copy()
    n = 3
    Q = 1 << (bits - 1)
    while Q > 1:
        P = Q - 1
        for i in range(n):
            mask = (X[:, i] & Q) != 0
            X[mask, 0] ^= P
            nm = ~mask
            t = (X[nm, 0] ^ X[nm, i]) & P
            X[nm, 0] ^= t
            X[nm, i] ^= t
        Q >>= 1
    for i in range(1, n):
        X[:, i] ^= X[:, i - 1]
    t = np.zeros(len(X), np.int64)
    Q = 1 << (bits - 1)
    while Q > 1:
        mask = (X[:, n - 1] & Q) != 0
        t[mask] ^= Q - 1
        Q >>= 1
    for i in range(n):
        X[:, i] ^= t
    key = np.zeros(len(X), np.int64)
    for b in range(bits - 1, -1, -1):
        for i in range(n):
            key = (key << 1) | ((X[:, i] >> b) & 1)
    return key


def _cells(pts):
    """Grid cell index per axis via the quantile boundaries."""
    return np.stack([np.searchsorted(QS_IN, pts[:, d]) for d in range(3)], 1)


def _prep_job(P, Q_pts, Lp, Rq):
    """Host index build for one (rows=P, candidates=Q_pts) job.

    Returns lhsT [32, N], rhsb [32, NBLK*W], row_order, r_cover (sorted order),
    and a bool mask (sorted order) of rows that must be host re-evaluated
    because their block's candidate list overflowed W or was empty.
    """
    n = len(P)
    cP = _cells(P)
    order = np.argsort(_hilbert_key(cP), kind="stable")
    Ps, cPs = P[order], cP[order]

    cQ = _cells(Q_pts)
    qcid = (cQ[:, 0] * G + cQ[:, 1]) * G + cQ[:, 2]
    qorder = np.argsort(qcid, kind="stable")
    cell_starts = np.searchsorted(qcid[qorder], np.arange(G ** 3 + 1))

    # negated planes: PSUM accumulates -||p-q||^2 so every reduction is a max
    lhsT = np.ascontiguousarray(-Lp[:, order])
    rhsb = np.empty((KROWS, NBLK * W), BF16)
    Rq_sorted = np.ascontiguousarray(Rq[:, qorder])
    forced = np.zeros(n, bool)

    lo_b = QS[np.maximum(cPs - 1, 0)]
    hi_b = QS[np.minimum(cPs + 2, G)]
    r_cover = np.minimum(Ps - lo_b, hi_b - Ps).min(1)

    for i in range(NBLK):
        sl = slice(i * 128, (i + 1) * 128)
        cc = cPs[sl]
        ucells = np.unique((cc[:, 0] * G + cc[:, 1]) * G + cc[:, 2])
        ux, uy, uz = ucells // (G * G), (ucells // G) % G, ucells % G
        ring = set()
        for dx in (-1, 0, 1):
            for dy in (-1, 0, 1):
                for dz in (-1, 0, 1):
                    nx, ny, nz = ux + dx, uy + dy, uz + dz
                    ok = (nx >= 0) & (nx < G) & (ny >= 0) & (ny < G) & (nz >= 0) & (nz < G)
                    ring.update(((nx[ok] * G + ny[ok]) * G + nz[ok]).tolist())
        segs = [np.arange(cell_starts[c], cell_starts[c + 1]) for c in sorted(ring)]
        cand = np.concatenate(segs) if segs else np.zeros(0, np.int64)
        if len(cand) == 0:
            forced[sl] = True
            cand = np.zeros(1, np.int64)
        elif len(cand) > W:
            forced[sl] = True
            cand = cand[:W]
        if len(cand) < W:
            cand = np.concatenate([cand, np.broadcast_to(cand[0], W - len(cand))])
        rhsb[:, i * W:(i + 1) * W] = Rq_sorted[:, cand]
    return lhsT, rhsb, order, r_cover, forced


def _build_program():
    """Build the SPMD Tile program once. Returns the finalized Bass object."""
    import concourse.bacc as bacc
    import concourse.tile as tile
    from concourse import mybir

    nc = bacc.Bacc("TRN2", target_bir_lowering=False, debug=False, num_devices=NCORES)

    lhsT_d = nc.dram_tensor("lhsT", [KROWS, N], mybir.dt.bfloat16, kind="ExternalInput")
    rhsb_d = nc.dram_tensor("rhsb", [KROWS, NBLK * W], mybir.dt.bfloat16, kind="ExternalInput")
    out_d = nc.dram_tensor("out", [128, NBLK], mybir.dt.float32, kind="ExternalOutput")

    with tile.TileContext(nc) as tc:
        with (
            tc.tile_pool(name="weights", bufs=1) as wpool,
            tc.tile_pool(name="psum", bufs=6, space="PSUM") as pspool,
            tc.tile_pool(name="evac", bufs=2) as epool,
            tc.tile_pool(name="outp", bufs=1) as opool,
        ):
            lhsT_sb = wpool.tile([KROWS, N], mybir.dt.bfloat16)
            rhsb_sb = wpool.tile([KROWS, NBLK * W], mybir.dt.bfloat16)
            nc.sync.dma_start(out=lhsT_sb, in_=lhsT_d[:, :])
            # chunked rhs load so early blocks can start before the tail lands
            rchunk = NBLK * W // 4
            for c in range(4):
                nc.sync.dma_start(
                    out=rhsb_sb[:, c * rchunk:(c + 1) * rchunk],
                    in_=rhsb_d[:, c * rchunk:(c + 1) * rchunk],
                )

            outacc = opool.tile([128, NBLK], mybir.dt.float32)

            for g in range(NGRP):
                evg = epool.tile([128, GROUP, W], mybir.dt.bfloat16)
                for j in range(GROUP):
                    i = g * GROUP + j
                    ps = pspool.tile([128, W], mybir.dt.float32)
                    nc.tensor.matmul(
                        ps,
                        lhsT_sb[:, i * 128:(i + 1) * 128],
                        rhsb_sb[:, i * W:(i + 1) * W],
                        start=True,
                        stop=True,
                    )
                    if ACT_GROUP[g]:
                        nc.scalar.activation(evg[:, j, :], ps, mybir.ActivationFunctionType.Copy)
                    else:
                        nc.vector.tensor_copy(evg[:, j, :], ps)
                    # multi-window pool is rejected by codegen: one pool per block
                    nc.vector.pool_max(outacc[:, i:i + 1], evg[:, j, :])
            nc.sync.dma_start(out=out_d[:, :], in_=outacc)

    nc.compile()
    return nc


def _get_program():
    global _PROGRAM
    if _PROGRAM is None:
        _PROGRAM = _build_program()
    return _PROGRAM


def build_in_maps(source, target):
    """Host prep: returns (in_maps, meta) where meta holds per-job unsort info."""
    in_maps, meta = [], []
    for b in range(B):
        L, R = _build_planes(source[b], target[b])      # rows=src planes, cand=tgt planes
        L2, R2 = _build_planes(target[b], source[b])    # rows=tgt planes, cand=src planes
        for direction in (0, 1):
            if direction == 0:
                lhsT, rhsb, order, r_cover, forced = _prep_job(source[b], target[b], L, R)
            else:
                lhsT, rhsb, order, r_cover, forced = _prep_job(target[b], source[b], L2, R2)
            in_maps.append({"lhsT": lhsT, "rhsb": rhsb})
            meta.append((b, direction, order, r_cover, forced))
    return in_maps, meta


def _exact_minsq_fp64(pts, others):
    """Exact (fp64) min squared distance from each of pts to the set others."""
    p = pts.astype(np.float64)
    o = others.astype(np.float64)
    no = (o * o).sum(1)
    out = np.empty(len(p), np.float64)
    for i0 in range(0, len(p), 2048):
        pp = p[i0:i0 + 2048]
        sq = ((pp * pp).sum(1))[:, None] + no[None, :] - 2.0 * (pp @ o.T)
        out[i0:i0 + 2048] = sq.min(1)
    return np.maximum(out, 0.0)


def kernel(source, target, weights):
    from concourse.bass_utils import run_bass_kernel_spmd

    source = np.asarray(source)
    target = np.asarray(target)
    weights = np.asarray(weights)

    in_maps, meta = build_in_maps(source, target)

    nc = _get_program()
    res = None
    last_err = None
    for attempt in range(3):
        try:
            res = run_bass_kernel_spmd(nc, in_maps, list(range(NCORES))).results
            break
        except Exception as e:  # transient device wedge: retry
            last_err = e
            import time as _time

            _time.sleep(5.0 * (attempt + 1))
    if res is None:
        raise last_err

    s_minsq = np.empty((B, N), np.float64)
    t_minsq = np.empty((B, M), np.float64)
    for j in range(NCORES):
        b, direction, order, r_cover, forced = meta[j]
        wmin_sorted = np.maximum(-res[j]["out"].T.reshape(-1).astype(np.float64), 0.0)
        # certificate: exact unless min reaches the covered-region boundary
        bad = forced | (np.sqrt(wmin_sorted) >= CERT_MARGIN * r_cover) | (wmin_sorted < SMALL_SQ_THRESH)
        pts = source[b] if direction == 0 else target[b]
        others = target[b] if direction == 0 else source[b]
        bad_rows = order[np.flatnonzero(bad)]
        full = np.empty(len(pts), np.float64)
        full[order] = wmin_sorted
        if len(bad_rows):
            full[bad_rows] = _exact_minsq_fp64(pts[bad_rows], others)
        if direction == 0:
            s_minsq[b] = full
        else:
            t_minsq[b] = full

    fwd = float((np.sqrt(s_minsq + EPS) * weights.astype(np.float64)).mean())
    bwd = float(np.sqrt(t_minsq + EPS).mean())
    return np.float32(fwd + bwd)


# revision 16
# speedup vs baseline: 10.2341x; 1.1796x over previous
"""Chamfer distance (symmetric, weighted forward) on 8 Trainium2 NeuronCores.

Strategy: grid-pruned nearest-neighbor search ("cell lists").
----------------------------------------------------------------
Both point sets of a batch are binned into an 18^3 rectilinear grid whose
boundaries are N(0,1) quantiles (coords are iid normal => near-uniform cell
occupancy). Points are ordered by the Hilbert index of their cell; each block
of 128 rows scans only the targets in the 1-ring of the block's cells
(host-gathered, padded to a static W=512 window). Each of the 8 cores handles
one (batch, direction) job: 64 blocks x [K=32] x [128 rows x 512 candidates]
augmented matmuls (negated, so PSUM holds -||s-t||^2; fp32 operands split into
bf16 planes, products exact in fp32), then one DVE pool_max per block reads
PSUM directly and writes the per-row max of -sq (= -min sq).

Exactness: a windowed min is provably exact when it is smaller than the
distance from the point to the boundary of its own cell's 1-ring (r_cover).
The host re-evaluates (fp64) the few rows failing that certificate, rows of
overflowing/empty blocks, and near-zero mins where sqrt amplifies fp noise.
This holds for ANY input data, not just the benchmarked distribution.
"""

import os
import sys

import numpy as np

for _p in ("/root/.axon_site", "/root/.axon_site/_ro/trn_rl_repo", "/root/.axon_site/_ro/pypackages"):
    if os.path.isdir(_p) and _p not in sys.path:
        sys.path.append(_p)

import ml_dtypes

BF16 = ml_dtypes.bfloat16

# Problem constants (hardcoded per spec)
B = 4
N = 8192          # sources per batch
M = 8192          # targets per batch
NCORES = 8
KROWS = 32        # bf16 planes of the augmented matmul
NBLK = N // 128   # 64 row blocks per job
W = 512           # candidate window per block
G = 18            # grid resolution per axis
HBITS = 5
EPS = 1e-8
SMALL_SQ_THRESH = 4e-4
CERT_MARGIN = 0.98

# Interior N(0,1) quantile boundaries of the G=18 grid (exact same grid the
# certificate radii are computed from).
QS_IN = np.array([
    -1.59321882, -1.22064035, -0.967421566, -0.764709674, -0.589455798,
    -0.430727299, -0.282216147, -0.139710299, 0.0, 0.139710299, 0.282216147,
    0.430727299, 0.589455798, 0.764709674, 0.967421566, 1.22064035, 1.59321882,
], np.float64)
QS = np.concatenate([[-np.inf], QS_IN, [np.inf]])  # length G+1



_PROGRAM = None  # cached compiled Bass program


def _splitn(x, n):
    """Split fp64 array into n bf16 planes summing (to ~8n bits) to x."""
    x = x.astype(np.float64)
    out = []
    for _ in range(n):
        a = x.astype(BF16)
        out.append(a)
        x = x - a.astype(np.float64)
    return out


def _build_planes(src_b, tgt_b):
    """Augmented K=32 bf16 planes: sum_k L[k,n] R[k,m] == ||s_n - t_m||^2."""
    sa, sb, sc = _splitn(-2.0 * src_b.astype(np.float64), 3)
    ta, tb, tc = _splitn(tgt_b.astype(np.float64), 3)
    ns = (src_b.astype(np.float64) ** 2).sum(1)
    nt = (tgt_b.astype(np.float64) ** 2).sum(1)
    nss = _splitn(ns, 4)
    nts = _splitn(nt, 4)
    one_s = np.ones(ns.shape, BF16)
    one_t = np.ones(nt.shape, BF16)
    Ls, Rs = [], []
    for k in range(3):
        for (u, v) in [(sa, ta), (sa, tb), (sa, tc), (sb, ta), (sb, tb), (sb, tc), (sc, ta), (sc, tb)]:
            Ls.append(u[:, k])
            Rs.append(v[:, k])
    for u in nss:
        Ls.append(u)
        Rs.append(one_t)
    for v in nts:
        Ls.append(one_s)
        Rs.append(v)
    L = np.ascontiguousarray(np.stack(Ls, 0).astype(BF16))
    R = np.ascontiguousarray(np.stack(Rs, 0).astype(BF16))
    return L, R


def _hilbert_key(c, bits=HBITS):
    """Hilbert index of integer 3d cells (Skilling transpose algorithm)."""
    X = c.astype(np.int64).copy()
    n = 3
    Q = 1 << (bits - 1)
    while Q > 1:
        P = Q - 1
        for i in range(n):
            mask = (X[:, i] & Q) != 0
            X[mask, 0] ^= P
            nm = ~mask
            t = (X[nm, 0] ^ X[nm, i]) & P
            X[nm, 0] ^= t
            X[nm, i] ^= t
        Q >>= 1
    for i in range(1, n):
        X[:, i] ^= X[:, i - 1]
    t = np.zeros(len(X), np.int64)
    Q = 1 << (bits - 1)
    while Q > 1:
        mask = (X[:, n - 1] & Q) != 0
        t[mask] ^= Q - 1
        Q >>= 1
    for i in range(n):
        X[:, i] ^= t
    key = np.zeros(len(X), np.int64)
    for b in range(bits - 1, -1, -1):
        for i in range(n):
            key = (key << 1) | ((X[:, i] >> b) & 1)
    return key


def _cells(pts):
    """Grid cell index per axis via the quantile boundaries."""
    return np.stack([np.searchsorted(QS_IN, pts[:, d]) for d in range(3)], 1)


def _prep_job(P, Q_pts, Lp, Rq):
    """Host index build for one (rows=P, candidates=Q_pts) job.

    Returns lhsT [32, N], rhsb [32, NBLK*W], row_order, r_cover (sorted order),
    and a bool mask (sorted order) of rows that must be host re-evaluated
    because their block's candidate list overflowed W or was empty.
    """
    n = len(P)
    cP = _cells(P)
    order = np.argsort(_hilbert_key(cP), kind="stable")
    Ps, cPs = P[order], cP[order]

    cQ = _cells(Q_pts)
    qcid = (cQ[:, 0] * G + cQ[:, 1]) * G + cQ[:, 2]
    qorder = np.argsort(qcid, kind="stable")
    cell_starts = np.searchsorted(qcid[qorder], np.arange(G ** 3 + 1))

    # negated planes: PSUM accumulates -||p-q||^2 so every reduction is a max
    lhsT = np.ascontiguousarray(-Lp[:, order])
    rhsb = np.empty((KROWS, NBLK * W), BF16)
    Rq_sorted = np.ascontiguousarray(Rq[:, qorder])
    forced = np.zeros(n, bool)

    lo_b = QS[np.maximum(cPs - 1, 0)]
    hi_b = QS[np.minimum(cPs + 2, G)]
    r_cover = np.minimum(Ps - lo_b, hi_b - Ps).min(1)

    for i in range(NBLK):
        sl = slice(i * 128, (i + 1) * 128)
        cc = cPs[sl]
        ucells = np.unique((cc[:, 0] * G + cc[:, 1]) * G + cc[:, 2])
        ux, uy, uz = ucells // (G * G), (ucells // G) % G, ucells % G
        ring = set()
        for dx in (-1, 0, 1):
            for dy in (-1, 0, 1):
                for dz in (-1, 0, 1):
                    nx, ny, nz = ux + dx, uy + dy, uz + dz
                    ok = (nx >= 0) & (nx < G) & (ny >= 0) & (ny < G) & (nz >= 0) & (nz < G)
                    ring.update(((nx[ok] * G + ny[ok]) * G + nz[ok]).tolist())
        segs = [np.arange(cell_starts[c], cell_starts[c + 1]) for c in sorted(ring)]
        cand = np.concatenate(segs) if segs else np.zeros(0, np.int64)
        if len(cand) == 0:
            forced[sl] = True
            cand = np.zeros(1, np.int64)
        elif len(cand) > W:
            forced[sl] = True
            cand = cand[:W]
        if len(cand) < W:
            cand = np.concatenate([cand, np.broadcast_to(cand[0], W - len(cand))])
        rhsb[:, i * W:(i + 1) * W] = Rq_sorted[:, cand]
    return lhsT, rhsb, order, r_cover, forced


def _build_program():
    """Build the SPMD Tile program once. Returns the finalized Bass object."""
    import concourse.bacc as bacc
    import concourse.tile as tile
    from concourse import mybir

    nc = bacc.Bacc("TRN2", target_bir_lowering=False, debug=False, num_devices=NCORES)

    lhsT_d = nc.dram_tensor("lhsT", [KROWS, N], mybir.dt.bfloat16, kind="ExternalInput")
    rhsb_d = nc.dram_tensor("rhsb", [KROWS, NBLK * W], mybir.dt.bfloat16, kind="ExternalInput")
    out_d = nc.dram_tensor("out", [128, NBLK], mybir.dt.float32, kind="ExternalOutput")

    with tile.TileContext(nc) as tc:
        with (
            tc.tile_pool(name="weights", bufs=1) as wpool,
            tc.tile_pool(name="psum", bufs=8, space="PSUM") as pspool,
            tc.tile_pool(name="outp", bufs=1) as opool,
        ):
            lhsT_sb = wpool.tile([KROWS, N], mybir.dt.bfloat16)
            rhsb_sb = wpool.tile([KROWS, NBLK * W], mybir.dt.bfloat16)
            nc.sync.dma_start(out=lhsT_sb, in_=lhsT_d[:, :])
            # chunked rhs load so early blocks can start before the tail lands
            rchunk = NBLK * W // 4
            for c in range(4):
                nc.sync.dma_start(
                    out=rhsb_sb[:, c * rchunk:(c + 1) * rchunk],
                    in_=rhsb_d[:, c * rchunk:(c + 1) * rchunk],
                )

            outacc = opool.tile([128, NBLK], mybir.dt.float32)

            for i in range(NBLK):
                ps = pspool.tile([128, W], mybir.dt.float32)
                nc.tensor.matmul(
                    ps,
                    lhsT_sb[:, i * 128:(i + 1) * 128],
                    rhsb_sb[:, i * W:(i + 1) * W],
                    start=True,
                    stop=True,
                )
                # pool reads PSUM directly (runs at the same 1x rate as a
                # pool from SBUF would, so the evacuation pass is pure waste)
                nc.vector.pool_max(outacc[:, i:i + 1], ps)
            nc.sync.dma_start(out=out_d[:, :], in_=outacc)

    nc.compile()
    return nc


def _get_program():
    global _PROGRAM
    if _PROGRAM is None:
        _PROGRAM = _build_program()
    return _PROGRAM


def build_in_maps(source, target):
    """Host prep: returns (in_maps, meta) where meta holds per-job unsort info."""
    in_maps, meta = [], []
    for b in range(B):
        L, R = _build_planes(source[b], target[b])      # rows=src planes, cand=tgt planes
        L2, R2 = _build_planes(target[b], source[b])    # rows=tgt planes, cand=src planes
        for direction in (0, 1):
            if direction == 0:
                lhsT, rhsb, order, r_cover, forced = _prep_job(source[b], target[b], L, R)
            else:
                lhsT, rhsb, order, r_cover, forced = _prep_job(target[b], source[b], L2, R2)
            in_maps.append({"lhsT": lhsT, "rhsb": rhsb})
            meta.append((b, direction, order, r_cover, forced))
    return in_maps, meta


def _exact_minsq_fp64(pts, others):
    """Exact (fp64) min squared distance from each of pts to the set others."""
    p = pts.astype(np.float64)
    o = others.astype(np.float64)
    no = (o * o).sum(1)
    out = np.empty(len(p), np.float64)
    for i0 in range(0, len(p), 2048):
        pp = p[i0:i0 + 2048]
        sq = ((pp * pp).sum(1))[:, None] + no[None, :] - 2.0 * (pp @ o.T)
        out[i0:i0 + 2048] = sq.min(1)
    return np.maximum(out, 0.0)


def kernel(source, target, weights):
    from concourse.bass_utils import run_bass_kernel_spmd

    source = np.asarray(source)
    target = np.asarray(target)
    weights = np.asarray(weights)

    in_maps, meta = build_in_maps(source, target)

    nc = _get_program()
    res = None
    last_err = None
    for attempt in range(3):
        try:
            res = run_bass_kernel_spmd(nc, in_maps, list(range(NCORES))).results
            break
        except Exception as e:  # transient device wedge: retry
            last_err = e
            import time as _time

            _time.sleep(5.0 * (attempt + 1))
    if res is None:
        raise last_err

    s_minsq = np.empty((B, N), np.float64)
    t_minsq = np.empty((B, M), np.float64)
    for j in range(NCORES):
        b, direction, order, r_cover, forced = meta[j]
        wmin_sorted = np.maximum(-res[j]["out"].T.reshape(-1).astype(np.float64), 0.0)
        # certificate: exact unless min reaches the covered-region boundary
        bad = forced | (np.sqrt(wmin_sorted) >= CERT_MARGIN * r_cover) | (wmin_sorted < SMALL_SQ_THRESH)
        pts = source[b] if direction == 0 else target[b]
        others = target[b] if direction == 0 else source[b]
        bad_rows = order[np.flatnonzero(bad)]
        full = np.empty(len(pts), np.float64)
        full[order] = wmin_sorted
        if len(bad_rows):
            full[bad_rows] = _exact_minsq_fp64(pts[bad_rows], others)
        if direction == 0:
            s_minsq[b] = full
        else:
            t_minsq[b] = full

    fwd = float((np.sqrt(s_minsq + EPS) * weights.astype(np.float64)).mean())
    bwd = float(np.sqrt(t_minsq + EPS).mean())
    return np.float32(fwd + bwd)


# revision 20
# speedup vs baseline: 11.9448x; 1.1672x over previous
"""Chamfer distance (symmetric, weighted forward) on 8 Trainium2 NeuronCores.

Strategy: grid-pruned nearest-neighbor search ("cell lists").
----------------------------------------------------------------
Both point sets of a batch are binned into an 18^3 rectilinear grid whose
boundaries are N(0,1) quantiles (coords are iid normal => near-uniform cell
occupancy). Points are ordered by the Hilbert index of their cell; each block
of 128 rows scans only the targets in the 1-ring of the block's cells
(host-gathered, padded to a static W=512 window). Each of the 8 cores handles
one (batch, direction) job: 64 blocks x [K=32] x [128 rows x 512 candidates]
augmented matmuls (negated, so PSUM holds -||s-t||^2; fp32 operands split into
bf16 planes, products exact in fp32), then one DVE pool_max per block reads
PSUM directly and writes the per-row max of -sq (= -min sq).

Exactness: a windowed min is provably exact when it is smaller than the
distance from the point to the boundary of its own cell's 1-ring (r_cover).
The host re-evaluates (fp64) the few rows failing that certificate, rows of
overflowing/empty blocks, and near-zero mins where sqrt amplifies fp noise.
This holds for ANY input data, not just the benchmarked distribution.
"""

import os
import sys

import numpy as np

for _p in ("/root/.axon_site", "/root/.axon_site/_ro/trn_rl_repo", "/root/.axon_site/_ro/pypackages"):
    if os.path.isdir(_p) and _p not in sys.path:
        sys.path.append(_p)

import ml_dtypes

BF16 = ml_dtypes.bfloat16

# Problem constants (hardcoded per spec)
B = 4
N = 8192          # sources per batch
M = 8192          # targets per batch
NCORES = 8
KROWS = 32        # bf16 planes of the augmented matmul
NBLK = N // 128   # 64 row blocks per job
G = 18            # grid resolution per axis

# Static per-slot candidate-window widths (descending). The host assigns row
# blocks to slots by descending candidate count, so the width schedule only
# needs to cover the sorted count curve (+margin) instead of a flat maximum.
# Blocks whose candidates overflow their slot are truncated and re-evaluated
# exactly on the host, so correctness never depends on this schedule.
TIER_W = [
    512, 496, 496, 496, 496, 480, 480, 480, 464, 464, 464, 464, 464, 432,
    432, 432, 432, 432, 432, 432, 416, 416, 416, 416, 416, 416, 400, 400,
    400, 400, 400, 384, 384, 384, 384, 384, 384, 384, 368, 368, 368, 368,
    368, 368, 368, 368, 352, 352, 352, 352, 336, 336, 336, 336, 336, 336,
    336, 336, 320, 320, 320, 304, 304, 288,
]
TIER_OFF = np.concatenate([[0], np.cumsum(TIER_W)]).astype(int)
TOTW = int(TIER_OFF[-1])
HBITS = 5
EPS = 1e-8
SMALL_SQ_THRESH = 4e-4
CERT_MARGIN = 0.98

# Interior N(0,1) quantile boundaries of the G=18 grid (exact same grid the
# certificate radii are computed from).
QS_IN = np.array([
    -1.59321882, -1.22064035, -0.967421566, -0.764709674, -0.589455798,
    -0.430727299, -0.282216147, -0.139710299, 0.0, 0.139710299, 0.282216147,
    0.430727299, 0.589455798, 0.764709674, 0.967421566, 1.22064035, 1.59321882,
], np.float64)
QS = np.concatenate([[-np.inf], QS_IN, [np.inf]])  # length G+1



_PROGRAM = None  # cached compiled Bass program


def _splitn(x, n):
    """Split fp64 array into n bf16 planes summing (to ~8n bits) to x."""
    x = x.astype(np.float64)
    out = []
    for _ in range(n):
        a = x.astype(BF16)
        out.append(a)
        x = x - a.astype(np.float64)
    return out


def _build_planes(src_b, tgt_b):
    """Augmented K=32 bf16 planes: sum_k L[k,n] R[k,m] == ||s_n - t_m||^2."""
    sa, sb, sc = _splitn(-2.0 * src_b.astype(np.float64), 3)
    ta, tb, tc = _splitn(tgt_b.astype(np.float64), 3)
    ns = (src_b.astype(np.float64) ** 2).sum(1)
    nt = (tgt_b.astype(np.float64) ** 2).sum(1)
    nss = _splitn(ns, 4)
    nts = _splitn(nt, 4)
    one_s = np.ones(ns.shape, BF16)
    one_t = np.ones(nt.shape, BF16)
    Ls, Rs = [], []
    for k in range(3):
        for (u, v) in [(sa, ta), (sa, tb), (sa, tc), (sb, ta), (sb, tb), (sb, tc), (sc, ta), (sc, tb)]:
            Ls.append(u[:, k])
            Rs.append(v[:, k])
    for u in nss:
        Ls.append(u)
        Rs.append(one_t)
    for v in nts:
        Ls.append(one_s)
        Rs.append(v)
    L = np.ascontiguousarray(np.stack(Ls, 0).astype(BF16))
    R = np.ascontiguousarray(np.stack(Rs, 0).astype(BF16))
    return L, R


def _hilbert_key(c, bits=HBITS):
    """Hilbert index of integer 3d cells (Skilling transpose algorithm)."""
    X = c.astype(np.int64).copy()
    n = 3
    Q = 1 << (bits - 1)
    while Q > 1:
        P = Q - 1
        for i in range(n):
            mask = (X[:, i] & Q) != 0
            X[mask, 0] ^= P
            nm = ~mask
            t = (X[nm, 0] ^ X[nm, i]) & P
            X[nm, 0] ^= t
            X[nm, i] ^= t
        Q >>= 1
    for i in range(1, n):
        X[:, i] ^= X[:, i - 1]
    t = np.zeros(len(X), np.int64)
    Q = 1 << (bits - 1)
    while Q > 1:
        mask = (X[:, n - 1] & Q) != 0
        t[mask] ^= Q - 1
        Q >>= 1
    for i in range(n):
        X[:, i] ^= t
    key = np.zeros(len(X), np.int64)
    for b in range(bits - 1, -1, -1):
        for i in range(n):
            key = (key << 1) | ((X[:, i] >> b) & 1)
    return key


def _cells(pts):
    """Grid cell index per axis via the quantile boundaries."""
    return np.stack([np.searchsorted(QS_IN, pts[:, d]) for d in range(3)], 1)


def _prep_job(P, Q_pts, Lp, Rq):
    """Host index build for one (rows=P, candidates=Q_pts) job.

    Returns lhsT [32, N], rhsb [32, TOTW], row_order (block-permuted so the
    s-th slot holds the block with the s-th largest candidate count), r_cover
    (in that order), and a bool mask of rows that must be host re-evaluated
    because their block's candidate list overflowed its slot or was empty.
    """
    n = len(P)
    cP = _cells(P)
    order = np.argsort(_hilbert_key(cP), kind="stable")
    cPs = cP[order]

    cQ = _cells(Q_pts)
    qcid = (cQ[:, 0] * G + cQ[:, 1]) * G + cQ[:, 2]
    qorder = np.argsort(qcid, kind="stable")
    cell_starts = np.searchsorted(qcid[qorder], np.arange(G ** 3 + 1))
    Rq_sorted = np.ascontiguousarray(Rq[:, qorder])

    # pass 1: candidate lists (1-ring of each block's occupied cells)
    cands = []
    for i in range(NBLK):
        cc = cPs[i * 128:(i + 1) * 128]
        ucells = np.unique((cc[:, 0] * G + cc[:, 1]) * G + cc[:, 2])
        ux, uy, uz = ucells // (G * G), (ucells // G) % G, ucells % G
        ring = set()
        for dx in (-1, 0, 1):
            for dy in (-1, 0, 1):
                for dz in (-1, 0, 1):
                    nx, ny, nz = ux + dx, uy + dy, uz + dz
                    ok = (nx >= 0) & (nx < G) & (ny >= 0) & (ny < G) & (nz >= 0) & (nz < G)
                    ring.update(((nx[ok] * G + ny[ok]) * G + nz[ok]).tolist())
        segs = [np.arange(cell_starts[c], cell_starts[c + 1]) for c in sorted(ring)]
        cands.append(np.concatenate(segs) if segs else np.zeros(0, np.int64))

    # pass 2: biggest blocks into the widest slots
    perm = np.argsort(-np.array([len(c) for c in cands]), kind="stable")
    order = np.concatenate([order[p * 128:(p + 1) * 128] for p in perm])
    Ps, cPs = P[order], cP[order]

    # negated planes: PSUM accumulates -||p-q||^2 so every reduction is a max
    lhsT = np.ascontiguousarray(-Lp[:, order])
    rhsb = np.empty((KROWS, TOTW), BF16)
    forced = np.zeros(n, bool)

    lo_b = QS[np.maximum(cPs - 1, 0)]
    hi_b = QS[np.minimum(cPs + 2, G)]
    r_cover = np.minimum(Ps - lo_b, hi_b - Ps).min(1)

    for s in range(NBLK):
        cand = cands[perm[s]]
        w = TIER_W[s]
        if len(cand) == 0:
            forced[s * 128:(s + 1) * 128] = True
            cand = np.zeros(1, np.int64)
        elif len(cand) > w:
            forced[s * 128:(s + 1) * 128] = True
            cand = cand[:w]
        if len(cand) < w:
            cand = np.concatenate([cand, np.broadcast_to(cand[0], w - len(cand))])
        rhsb[:, TIER_OFF[s]:TIER_OFF[s + 1]] = Rq_sorted[:, cand]
    return lhsT, rhsb, order, r_cover, forced


def _build_program():
    """Build the SPMD Tile program once. Returns the finalized Bass object."""
    import concourse.bacc as bacc
    import concourse.tile as tile
    from concourse import mybir

    nc = bacc.Bacc("TRN2", target_bir_lowering=False, debug=False, num_devices=NCORES)

    lhsT_d = nc.dram_tensor("lhsT", [KROWS, N], mybir.dt.bfloat16, kind="ExternalInput")
    rhsb_d = nc.dram_tensor("rhsb", [KROWS, TOTW], mybir.dt.bfloat16, kind="ExternalInput")
    out_d = nc.dram_tensor("out", [128, NBLK], mybir.dt.float32, kind="ExternalOutput")

    with tile.TileContext(nc) as tc:
        with (
            tc.tile_pool(name="weights", bufs=1) as wpool,
            tc.tile_pool(name="psum", bufs=8, space="PSUM") as pspool,
            tc.tile_pool(name="outp", bufs=1) as opool,
        ):
            lhsT_sb = wpool.tile([KROWS, N], mybir.dt.bfloat16)
            rhsb_sb = wpool.tile([KROWS, TOTW], mybir.dt.bfloat16)
            nc.sync.dma_start(out=lhsT_sb, in_=lhsT_d[:, :])
            # chunked rhs load so early blocks can start before the tail lands
            for c in range(0, NBLK, 8):
                nc.sync.dma_start(
                    out=rhsb_sb[:, TIER_OFF[c]:TIER_OFF[c + 8]],
                    in_=rhsb_d[:, TIER_OFF[c]:TIER_OFF[c + 8]],
                )

            outacc = opool.tile([128, NBLK], mybir.dt.float32)

            for s in range(NBLK):
                w = TIER_W[s]
                ps = pspool.tile([128, w], mybir.dt.float32)
                nc.tensor.matmul(
                    ps,
                    lhsT_sb[:, s * 128:(s + 1) * 128],
                    rhsb_sb[:, TIER_OFF[s]:TIER_OFF[s + 1]],
                    start=True,
                    stop=True,
                )
                # pool reads PSUM directly (runs at the same 1x rate as a
                # pool from SBUF would, so the evacuation pass is pure waste)
                nc.vector.pool_max(outacc[:, s:s + 1], ps)
            nc.sync.dma_start(out=out_d[:, :], in_=outacc)

    nc.compile()
    return nc


def _get_program():
    global _PROGRAM
    if _PROGRAM is None:
        _PROGRAM = _build_program()
    return _PROGRAM


def build_in_maps(source, target):
    """Host prep: returns (in_maps, meta) where meta holds per-job unsort info."""
    in_maps, meta = [], []
    for b in range(B):
        L, R = _build_planes(source[b], target[b])      # rows=src planes, cand=tgt planes
        L2, R2 = _build_planes(target[b], source[b])    # rows=tgt planes, cand=src planes
        for direction in (0, 1):
            if direction == 0:
                lhsT, rhsb, order, r_cover, forced = _prep_job(source[b], target[b], L, R)
            else:
                lhsT, rhsb, order, r_cover, forced = _prep_job(target[b], source[b], L2, R2)
            in_maps.append({"lhsT": lhsT, "rhsb": rhsb})
            meta.append((b, direction, order, r_cover, forced))
    return in_maps, meta


def _exact_minsq_fp64(pts, others):
    """Exact (fp64) min squared distance from each of pts to the set others."""
    p = pts.astype(np.float64)
    o = others.astype(np.float64)
    no = (o * o).sum(1)
    out = np.empty(len(p), np.float64)
    for i0 in range(0, len(p), 2048):
        pp = p[i0:i0 + 2048]
        sq = ((pp * pp).sum(1))[:, None] + no[None, :] - 2.0 * (pp @ o.T)
        out[i0:i0 + 2048] = sq.min(1)
    return np.maximum(out, 0.0)


def kernel(source, target, weights):
    from concourse.bass_utils import run_bass_kernel_spmd

    source = np.asarray(source)
    target = np.asarray(target)
    weights = np.asarray(weights)

    in_maps, meta = build_in_maps(source, target)

    nc = _get_program()
    res = None
    last_err = None
    for attempt in range(3):
        try:
            res = run_bass_kernel_spmd(nc, in_maps, list(range(NCORES))).results
            break
        except Exception as e:  # transient device wedge: retry
            last_err = e
            import time as _time

            _time.sleep(5.0 * (attempt + 1))
    if res is None:
        raise last_err

    s_minsq = np.empty((B, N), np.float64)
    t_minsq = np.empty((B, M), np.float64)
    for j in range(NCORES):
        b, direction, order, r_cover, forced = meta[j]
        wmin_sorted = np.maximum(-res[j]["out"].T.reshape(-1).astype(np.float64), 0.0)
        # certificate: exact unless min reaches the covered-region boundary
        bad = forced | (np.sqrt(wmin_sorted) >= CERT_MARGIN * r_cover) | (wmin_sorted < SMALL_SQ_THRESH)
        pts = source[b] if direction == 0 else target[b]
        others = target[b] if direction == 0 else source[b]
        bad_rows = order[np.flatnonzero(bad)]
        full = np.empty(len(pts), np.float64)
        full[order] = wmin_sorted
        if len(bad_rows):
            full[bad_rows] = _exact_minsq_fp64(pts[bad_rows], others)
        if direction == 0:
            s_minsq[b] = full
        else:
            t_minsq[b] = full

    fwd = float((np.sqrt(s_minsq + EPS) * weights.astype(np.float64)).mean())
    bwd = float(np.sqrt(t_minsq + EPS).mean())
    return np.float32(fwd + bwd)


# revision 24
# speedup vs baseline: 12.7879x; 1.0706x over previous
"""Chamfer distance (symmetric, weighted forward) on 8 Trainium2 NeuronCores.

Strategy: grid-pruned nearest-neighbor search ("cell lists").
----------------------------------------------------------------
Both point sets of a batch are binned into a 20^3 rectilinear grid whose
boundaries are N(0,1) quantiles (coords are iid normal => near-uniform cell
occupancy). Points are ordered by the Hilbert index of their cell; each block
of 128 rows scans only the targets in the 1-ring of the block's cells
(host-gathered into per-slot windows of tiered static widths, biggest blocks
first). Each of the 8 cores handles one (batch, direction) job: 64 slots of
[K=32] x [128 rows x W_s candidates] augmented matmuls (negated, so PSUM
holds -||s-t||^2; fp32 operands split into bf16 planes, products exact in
fp32), then one DVE pool_max per slot reads PSUM directly and writes the
per-row max of -sq (= -min sq).

Exactness: a windowed min is provably exact when it is smaller than the
distance from the point to the boundary of its own cell's 1-ring (r_cover).
The host re-evaluates (fp64) the few rows failing that certificate, rows of
overflowing/empty blocks, and near-zero mins where sqrt amplifies fp noise.
This holds for ANY input data, not just the benchmarked distribution.
"""

import os
import sys

import numpy as np

for _p in ("/root/.axon_site", "/root/.axon_site/_ro/trn_rl_repo", "/root/.axon_site/_ro/pypackages"):
    if os.path.isdir(_p) and _p not in sys.path:
        sys.path.append(_p)

import ml_dtypes

BF16 = ml_dtypes.bfloat16

# Problem constants (hardcoded per spec)
B = 4
N = 8192          # sources per batch
M = 8192          # targets per batch
NCORES = 8
KROWS = 32        # bf16 planes of the augmented matmul
NBLK = N // 128   # 64 row blocks per job
G = 20            # grid resolution per axis

# Static per-slot candidate-window widths (descending). The host assigns row
# blocks to slots by descending candidate count, so the width schedule only
# needs to cover the sorted count curve (+margin) instead of a flat maximum.
# Blocks whose candidates overflow their slot are truncated and re-evaluated
# exactly on the host, so correctness never depends on this schedule.
TIER_W = [
    512, 464, 448, 432, 432, 432, 432, 400, 400, 400, 400, 400, 384, 384,
    384, 384, 384, 384, 384, 384, 368, 368, 368, 368, 352, 352, 352, 352,
    352, 352, 352, 336, 336, 336, 336, 336, 336, 336, 336, 336, 320, 320,
    320, 320, 320, 320, 320, 320, 304, 304, 304, 304, 304, 304, 304, 288,
    288, 288, 288, 288, 272, 272, 256, 256,
]
TIER_OFF = np.concatenate([[0], np.cumsum(TIER_W)]).astype(int)
TOTW = int(TIER_OFF[-1])
HBITS = 5
EPS = 1e-8
SMALL_SQ_THRESH = 4e-4
CERT_MARGIN = 0.98

# Interior N(0,1) quantile boundaries of the G=20 grid (exact same grid the
# certificate radii are computed from).
QS_IN = np.array([
    -1.64485363, -1.28155157, -1.03643339, -0.841621234, -0.67448975,
    -0.524400513, -0.385320466, -0.253347103, -0.125661347, 0.0,
    0.125661347, 0.253347103, 0.385320466, 0.524400513, 0.67448975,
    0.841621234, 1.03643339, 1.28155157, 1.64485363,
], np.float64)
QS = np.concatenate([[-np.inf], QS_IN, [np.inf]])  # length G+1



_PROGRAM = None  # cached compiled Bass program


def _splitn(x, n):
    """Split fp64 array into n bf16 planes summing (to ~8n bits) to x."""
    x = x.astype(np.float64)
    out = []
    for _ in range(n):
        a = x.astype(BF16)
        out.append(a)
        x = x - a.astype(np.float64)
    return out


def _build_planes(src_b, tgt_b):
    """Augmented K=32 bf16 planes: sum_k L[k,n] R[k,m] == ||s_n - t_m||^2."""
    sa, sb, sc = _splitn(-2.0 * src_b.astype(np.float64), 3)
    ta, tb, tc = _splitn(tgt_b.astype(np.float64), 3)
    ns = (src_b.astype(np.float64) ** 2).sum(1)
    nt = (tgt_b.astype(np.float64) ** 2).sum(1)
    nss = _splitn(ns, 4)
    nts = _splitn(nt, 4)
    one_s = np.ones(ns.shape, BF16)
    one_t = np.ones(nt.shape, BF16)
    Ls, Rs = [], []
    for k in range(3):
        for (u, v) in [(sa, ta), (sa, tb), (sa, tc), (sb, ta), (sb, tb), (sb, tc), (sc, ta), (sc, tb)]:
            Ls.append(u[:, k])
            Rs.append(v[:, k])
    for u in nss:
        Ls.append(u)
        Rs.append(one_t)
    for v in nts:
        Ls.append(one_s)
        Rs.append(v)
    L = np.ascontiguousarray(np.stack(Ls, 0).astype(BF16))
    R = np.ascontiguousarray(np.stack(Rs, 0).astype(BF16))
    return L, R


def _hilbert_key(c, bits=HBITS):
    """Hilbert index of integer 3d cells (Skilling transpose algorithm)."""
    X = c.astype(np.int64).copy()
    n = 3
    Q = 1 << (bits - 1)
    while Q > 1:
        P = Q - 1
        for i in range(n):
            mask = (X[:, i] & Q) != 0
            X[mask, 0] ^= P
            nm = ~mask
            t = (X[nm, 0] ^ X[nm, i]) & P
            X[nm, 0] ^= t
            X[nm, i] ^= t
        Q >>= 1
    for i in range(1, n):
        X[:, i] ^= X[:, i - 1]
    t = np.zeros(len(X), np.int64)
    Q = 1 << (bits - 1)
    while Q > 1:
        mask = (X[:, n - 1] & Q) != 0
        t[mask] ^= Q - 1
        Q >>= 1
    for i in range(n):
        X[:, i] ^= t
    key = np.zeros(len(X), np.int64)
    for b in range(bits - 1, -1, -1):
        for i in range(n):
            key = (key << 1) | ((X[:, i] >> b) & 1)
    return key


def _cells(pts):
    """Grid cell index per axis via the quantile boundaries."""
    return np.stack([np.searchsorted(QS_IN, pts[:, d]) for d in range(3)], 1)


def _prep_job(P, Q_pts, Lp, Rq):
    """Host index build for one (rows=P, candidates=Q_pts) job.

    Returns lhsT [32, N], rhsb [32, TOTW], row_order (block-permuted so the
    s-th slot holds the block with the s-th largest candidate count), r_cover
    (in that order), and a bool mask of rows that must be host re-evaluated
    because their block's candidate list overflowed its slot or was empty.
    """
    n = len(P)
    cP = _cells(P)
    order = np.argsort(_hilbert_key(cP), kind="stable")
    cPs = cP[order]

    cQ = _cells(Q_pts)
    qcid = (cQ[:, 0] * G + cQ[:, 1]) * G + cQ[:, 2]
    qorder = np.argsort(qcid, kind="stable")
    cell_starts = np.searchsorted(qcid[qorder], np.arange(G ** 3 + 1))
    Rq_sorted = np.ascontiguousarray(Rq[:, qorder])

    # pass 1: candidate lists (1-ring of each block's occupied cells)
    cands = []
    for i in range(NBLK):
        cc = cPs[i * 128:(i + 1) * 128]
        ucells = np.unique((cc[:, 0] * G + cc[:, 1]) * G + cc[:, 2])
        ux, uy, uz = ucells // (G * G), (ucells // G) % G, ucells % G
        ring = set()
        for dx in (-1, 0, 1):
            for dy in (-1, 0, 1):
                for dz in (-1, 0, 1):
                    nx, ny, nz = ux + dx, uy + dy, uz + dz
                    ok = (nx >= 0) & (nx < G) & (ny >= 0) & (ny < G) & (nz >= 0) & (nz < G)
                    ring.update(((nx[ok] * G + ny[ok]) * G + nz[ok]).tolist())
        segs = [np.arange(cell_starts[c], cell_starts[c + 1]) for c in sorted(ring)]
        cands.append(np.concatenate(segs) if segs else np.zeros(0, np.int64))

    # pass 2: biggest blocks into the widest slots
    perm = np.argsort(-np.array([len(c) for c in cands]), kind="stable")
    order = np.concatenate([order[p * 128:(p + 1) * 128] for p in perm])
    Ps, cPs = P[order], cP[order]

    # negated planes: PSUM accumulates -||p-q||^2 so every reduction is a max
    lhsT = np.ascontiguousarray(-Lp[:, order])
    rhsb = np.empty((KROWS, TOTW), BF16)
    forced = np.zeros(n, bool)

    lo_b = QS[np.maximum(cPs - 1, 0)]
    hi_b = QS[np.minimum(cPs + 2, G)]
    r_cover = np.minimum(Ps - lo_b, hi_b - Ps).min(1)

    for s in range(NBLK):
        cand = cands[perm[s]]
        w = TIER_W[s]
        if len(cand) == 0:
            forced[s * 128:(s + 1) * 128] = True
            cand = np.zeros(1, np.int64)
        elif len(cand) > w:
            forced[s * 128:(s + 1) * 128] = True
            cand = cand[:w]
        if len(cand) < w:
            cand = np.concatenate([cand, np.broadcast_to(cand[0], w - len(cand))])
        rhsb[:, TIER_OFF[s]:TIER_OFF[s + 1]] = Rq_sorted[:, cand]
    return lhsT, rhsb, order, r_cover, forced


def _build_program():
    """Build the SPMD Tile program once. Returns the finalized Bass object."""
    import concourse.bacc as bacc
    import concourse.tile as tile
    from concourse import mybir

    nc = bacc.Bacc("TRN2", target_bir_lowering=False, debug=False, num_devices=NCORES)

    lhsT_d = nc.dram_tensor("lhsT", [KROWS, N], mybir.dt.bfloat16, kind="ExternalInput")
    rhsb_d = nc.dram_tensor("rhsb", [KROWS, TOTW], mybir.dt.bfloat16, kind="ExternalInput")
    out_d = nc.dram_tensor("out", [128, NBLK], mybir.dt.float32, kind="ExternalOutput")

    with tile.TileContext(nc) as tc:
        with (
            tc.tile_pool(name="weights", bufs=1) as wpool,
            tc.tile_pool(name="psum", bufs=8, space="PSUM") as pspool,
            tc.tile_pool(name="outp", bufs=1) as opool,
        ):
            lhsT_sb = wpool.tile([KROWS, N], mybir.dt.bfloat16)
            rhsb_sb = wpool.tile([KROWS, TOTW], mybir.dt.bfloat16)
            # first 16 blocks' weights land early so slot 0 can start sooner
            nc.sync.dma_start(out=lhsT_sb[:, :2048], in_=lhsT_d[:, :2048])
            nc.sync.dma_start(out=lhsT_sb[:, 2048:], in_=lhsT_d[:, 2048:])
            # chunked rhs load so early blocks can start before the tail lands
            for c in range(0, NBLK, 8):
                nc.sync.dma_start(
                    out=rhsb_sb[:, TIER_OFF[c]:TIER_OFF[c + 8]],
                    in_=rhsb_d[:, TIER_OFF[c]:TIER_OFF[c + 8]],
                )

            outacc = opool.tile([128, NBLK], mybir.dt.float32)

            for s in range(NBLK):
                w = TIER_W[s]
                ps = pspool.tile([128, w], mybir.dt.float32)
                nc.tensor.matmul(
                    ps,
                    lhsT_sb[:, s * 128:(s + 1) * 128],
                    rhsb_sb[:, TIER_OFF[s]:TIER_OFF[s + 1]],
                    start=True,
                    stop=True,
                )
                # pool reads PSUM directly (runs at the same 1x rate as a
                # pool from SBUF would, so the evacuation pass is pure waste)
                nc.vector.pool_max(outacc[:, s:s + 1], ps)
            nc.sync.dma_start(out=out_d[:, :], in_=outacc)

    nc.compile()
    return nc


def _get_program():
    global _PROGRAM
    if _PROGRAM is None:
        _PROGRAM = _build_program()
    return _PROGRAM


def build_in_maps(source, target):
    """Host prep: returns (in_maps, meta) where meta holds per-job unsort info."""
    in_maps, meta = [], []
    for b in range(B):
        L, R = _build_planes(source[b], target[b])      # rows=src planes, cand=tgt planes
        L2, R2 = _build_planes(target[b], source[b])    # rows=tgt planes, cand=src planes
        for direction in (0, 1):
            if direction == 0:
                lhsT, rhsb, order, r_cover, forced = _prep_job(source[b], target[b], L, R)
            else:
                lhsT, rhsb, order, r_cover, forced = _prep_job(target[b], source[b], L2, R2)
            in_maps.append({"lhsT": lhsT, "rhsb": rhsb})
            meta.append((b, direction, order, r_cover, forced))
    return in_maps, meta


def _exact_minsq_fp64(pts, others):
    """Exact (fp64) min squared distance from each of pts to the set others."""
    p = pts.astype(np.float64)
    o = others.astype(np.float64)
    no = (o * o).sum(1)
    out = np.empty(len(p), np.float64)
    for i0 in range(0, len(p), 2048):
        pp = p[i0:i0 + 2048]
        sq = ((pp * pp).sum(1))[:, None] + no[None, :] - 2.0 * (pp @ o.T)
        out[i0:i0 + 2048] = sq.min(1)
    return np.maximum(out, 0.0)


def kernel(source, target, weights):
    from concourse.bass_utils import run_bass_kernel_spmd

    source = np.asarray(source)
    target = np.asarray(target)
    weights = np.asarray(weights)

    in_maps, meta = build_in_maps(source, target)

    nc = _get_program()
    res = None
    last_err = None
    for attempt in range(3):
        try:
            res = run_bass_kernel_spmd(nc, in_maps, list(range(NCORES))).results
            break
        except Exception as e:  # transient device wedge: retry
            last_err = e
            import time as _time

            _time.sleep(5.0 * (attempt + 1))
    if res is None:
        raise last_err

    s_minsq = np.empty((B, N), np.float64)
    t_minsq = np.empty((B, M), np.float64)
    for j in range(NCORES):
        b, direction, order, r_cover, forced = meta[j]
        wmin_sorted = np.maximum(-res[j]["out"].T.reshape(-1).astype(np.float64), 0.0)
        # certificate: exact unless min reaches the covered-region boundary
        bad = forced | (np.sqrt(wmin_sorted) >= CERT_MARGIN * r_cover) | (wmin_sorted < SMALL_SQ_THRESH)
        pts = source[b] if direction == 0 else target[b]
        others = target[b] if direction == 0 else source[b]
        bad_rows = order[np.flatnonzero(bad)]
        full = np.empty(len(pts), np.float64)
        full[order] = wmin_sorted
        if len(bad_rows):
            full[bad_rows] = _exact_minsq_fp64(pts[bad_rows], others)
        if direction == 0:
            s_minsq[b] = full
        else:
            t_minsq[b] = full

    fwd = float((np.sqrt(s_minsq + EPS) * weights.astype(np.float64)).mean())
    bwd = float(np.sqrt(t_minsq + EPS).mean())
    return np.float32(fwd + bwd)


# revision 27
# speedup vs baseline: 13.2543x; 1.0365x over previous
"""Chamfer distance (symmetric, weighted forward) on 8 Trainium2 NeuronCores.

Strategy: grid-pruned nearest-neighbor search ("cell lists").
----------------------------------------------------------------
Both point sets of a batch are binned into a 20^3 rectilinear grid whose
boundaries are N(0,1) quantiles (coords are iid normal => near-uniform cell
occupancy). Points are ordered by the Hilbert index of their cell; each block
of 128 rows scans only the targets in the 1-ring of the block's cells
(host-gathered into per-slot windows of tiered static widths, biggest blocks
first). Each of the 8 cores handles one (batch, direction) job: 64 slots of
[K=32] x [128 rows x W_s candidates] augmented matmuls (negated, so PSUM
holds -||s-t||^2; fp32 operands split into bf16 planes, products exact in
fp32), then one DVE pool_max per slot reads PSUM directly and writes the
per-row max of -sq (= -min sq).

Exactness: a windowed min is provably exact when it is smaller than the
distance from the point to the boundary of its own cell's 1-ring (r_cover).
The host re-evaluates (fp64) the few rows failing that certificate, rows of
overflowing/empty blocks, and near-zero mins where sqrt amplifies fp noise.
This holds for ANY input data, not just the benchmarked distribution.
"""

import os
import sys

import numpy as np

for _p in ("/root/.axon_site", "/root/.axon_site/_ro/trn_rl_repo", "/root/.axon_site/_ro/pypackages"):
    if os.path.isdir(_p) and _p not in sys.path:
        sys.path.append(_p)

import ml_dtypes

BF16 = ml_dtypes.bfloat16

# Problem constants (hardcoded per spec)
B = 4
N = 8192          # sources per batch
M = 8192          # targets per batch
NCORES = 8
KROWS = 27        # bf16 planes of the augmented matmul
NBLK = N // 128   # 64 row blocks per job
G = 20            # grid resolution per axis

# Static per-slot candidate-window widths (descending). The host assigns row
# blocks to slots by descending candidate count, so the width schedule only
# needs to cover the sorted count curve (+margin) instead of a flat maximum.
# Blocks whose candidates overflow their slot are truncated and re-evaluated
# exactly on the host, so correctness never depends on this schedule.
TIER_W = [
    512, 464, 448, 432, 432, 432, 432, 400, 400, 400, 400, 400, 384, 384,
    384, 384, 384, 384, 384, 384, 368, 368, 368, 368, 352, 352, 352, 352,
    352, 352, 352, 336, 336, 336, 336, 336, 336, 336, 336, 336, 320, 320,
    320, 320, 320, 320, 320, 320, 304, 304, 304, 304, 304, 304, 304, 288,
    288, 288, 288, 288, 272, 272, 256, 256,
]
TIER_OFF = np.concatenate([[0], np.cumsum(TIER_W)]).astype(int)
TOTW = int(TIER_OFF[-1])
HBITS = 5
EPS = 1e-8
SMALL_SQ_THRESH = 4e-4
CERT_MARGIN = 0.98

# Interior N(0,1) quantile boundaries of the G=20 grid (exact same grid the
# certificate radii are computed from).
QS_IN = np.array([
    -1.64485363, -1.28155157, -1.03643339, -0.841621234, -0.67448975,
    -0.524400513, -0.385320466, -0.253347103, -0.125661347, 0.0,
    0.125661347, 0.253347103, 0.385320466, 0.524400513, 0.67448975,
    0.841621234, 1.03643339, 1.28155157, 1.64485363,
], np.float64)
QS = np.concatenate([[-np.inf], QS_IN, [np.inf]])  # length G+1



_PROGRAM = None  # cached compiled Bass program


def _splitn(x, n):
    """Split fp64 array into n bf16 planes summing (to ~8n bits) to x."""
    x = x.astype(np.float64)
    out = []
    for _ in range(n):
        a = x.astype(BF16)
        out.append(a)
        x = x - a.astype(np.float64)
    return out


def _build_planes(src_b, tgt_b):
    """Augmented K=27 bf16 planes: sum_k L[k,n] R[k,m] == ||s_n - t_m||^2.

    Dropped product planes (c*e, c*f) and 4th norm planes contribute
    O(2^-24)-relative terms, far below the 2e-2 tolerance and the host
    re-evaluation thresholds.
    """
    sa, sb, sc = _splitn(-2.0 * src_b.astype(np.float64), 3)
    ta, tb, tc = _splitn(tgt_b.astype(np.float64), 3)
    ns = (src_b.astype(np.float64) ** 2).sum(1)
    nt = (tgt_b.astype(np.float64) ** 2).sum(1)
    nss = _splitn(ns, 3)
    nts = _splitn(nt, 3)
    one_s = np.ones(ns.shape, BF16)
    one_t = np.ones(nt.shape, BF16)
    Ls, Rs = [], []
    for k in range(3):
        for (u, v) in [(sa, ta), (sa, tb), (sa, tc), (sb, ta), (sb, tb), (sb, tc), (sc, ta)]:
            Ls.append(u[:, k])
            Rs.append(v[:, k])
    for u in nss:
        Ls.append(u)
        Rs.append(one_t)
    for v in nts:
        Ls.append(one_s)
        Rs.append(v)
    L = np.ascontiguousarray(np.stack(Ls, 0).astype(BF16))
    R = np.ascontiguousarray(np.stack(Rs, 0).astype(BF16))
    return L, R


def _hilbert_key(c, bits=HBITS):
    """Hilbert index of integer 3d cells (Skilling transpose algorithm)."""
    X = c.astype(np.int64).copy()
    n = 3
    Q = 1 << (bits - 1)
    while Q > 1:
        P = Q - 1
        for i in range(n):
            mask = (X[:, i] & Q) != 0
            X[mask, 0] ^= P
            nm = ~mask
            t = (X[nm, 0] ^ X[nm, i]) & P
            X[nm, 0] ^= t
            X[nm, i] ^= t
        Q >>= 1
    for i in range(1, n):
        X[:, i] ^= X[:, i - 1]
    t = np.zeros(len(X), np.int64)
    Q = 1 << (bits - 1)
    while Q > 1:
        mask = (X[:, n - 1] & Q) != 0
        t[mask] ^= Q - 1
        Q >>= 1
    for i in range(n):
        X[:, i] ^= t
    key = np.zeros(len(X), np.int64)
    for b in range(bits - 1, -1, -1):
        for i in range(n):
            key = (key << 1) | ((X[:, i] >> b) & 1)
    return key


def _cells(pts):
    """Grid cell index per axis via the quantile boundaries."""
    return np.stack([np.searchsorted(QS_IN, pts[:, d]) for d in range(3)], 1)


def _prep_job(P, Q_pts, Lp, Rq):
    """Host index build for one (rows=P, candidates=Q_pts) job.

    Returns lhsT [32, N], rhsb [32, TOTW], row_order (block-permuted so the
    s-th slot holds the block with the s-th largest candidate count), r_cover
    (in that order), and a bool mask of rows that must be host re-evaluated
    because their block's candidate list overflowed its slot or was empty.
    """
    n = len(P)
    cP = _cells(P)
    order = np.argsort(_hilbert_key(cP), kind="stable")
    cPs = cP[order]

    cQ = _cells(Q_pts)
    qcid = (cQ[:, 0] * G + cQ[:, 1]) * G + cQ[:, 2]
    qorder = np.argsort(qcid, kind="stable")
    cell_starts = np.searchsorted(qcid[qorder], np.arange(G ** 3 + 1))
    Rq_sorted = np.ascontiguousarray(Rq[:, qorder])

    # pass 1: candidate lists (1-ring of each block's occupied cells)
    cands = []
    for i in range(NBLK):
        cc = cPs[i * 128:(i + 1) * 128]
        ucells = np.unique((cc[:, 0] * G + cc[:, 1]) * G + cc[:, 2])
        ux, uy, uz = ucells // (G * G), (ucells // G) % G, ucells % G
        ring = set()
        for dx in (-1, 0, 1):
            for dy in (-1, 0, 1):
                for dz in (-1, 0, 1):
                    nx, ny, nz = ux + dx, uy + dy, uz + dz
                    ok = (nx >= 0) & (nx < G) & (ny >= 0) & (ny < G) & (nz >= 0) & (nz < G)
                    ring.update(((nx[ok] * G + ny[ok]) * G + nz[ok]).tolist())
        segs = [np.arange(cell_starts[c], cell_starts[c + 1]) for c in sorted(ring)]
        cands.append(np.concatenate(segs) if segs else np.zeros(0, np.int64))

    # pass 2: biggest blocks into the widest slots
    perm = np.argsort(-np.array([len(c) for c in cands]), kind="stable")
    order = np.concatenate([order[p * 128:(p + 1) * 128] for p in perm])
    Ps, cPs = P[order], cP[order]

    # negated planes: PSUM accumulates -||p-q||^2 so every reduction is a max
    lhsT = np.ascontiguousarray(-Lp[:, order])
    rhsb = np.empty((KROWS, TOTW), BF16)
    forced = np.zeros(n, bool)

    lo_b = QS[np.maximum(cPs - 1, 0)]
    hi_b = QS[np.minimum(cPs + 2, G)]
    r_cover = np.minimum(Ps - lo_b, hi_b - Ps).min(1)

    for s in range(NBLK):
        cand = cands[perm[s]]
        w = TIER_W[s]
        if len(cand) == 0:
            forced[s * 128:(s + 1) * 128] = True
            cand = np.zeros(1, np.int64)
        elif len(cand) > w:
            forced[s * 128:(s + 1) * 128] = True
            cand = cand[:w]
        if len(cand) < w:
            cand = np.concatenate([cand, np.broadcast_to(cand[0], w - len(cand))])
        rhsb[:, TIER_OFF[s]:TIER_OFF[s + 1]] = Rq_sorted[:, cand]
    return lhsT, rhsb, order, r_cover, forced


def _build_program():
    """Build the SPMD Tile program once. Returns the finalized Bass object."""
    import concourse.bacc as bacc
    import concourse.tile as tile
    from concourse import mybir

    nc = bacc.Bacc("TRN2", target_bir_lowering=False, debug=False, num_devices=NCORES)

    lhsT_d = nc.dram_tensor("lhsT", [KROWS, N], mybir.dt.bfloat16, kind="ExternalInput")
    rhsb_d = nc.dram_tensor("rhsb", [KROWS, TOTW], mybir.dt.bfloat16, kind="ExternalInput")
    out_d = nc.dram_tensor("out", [128, NBLK], mybir.dt.float32, kind="ExternalOutput")

    with tile.TileContext(nc) as tc:
        with (
            tc.tile_pool(name="weights", bufs=1) as wpool,
            tc.tile_pool(name="psum", bufs=8, space="PSUM") as pspool,
            tc.tile_pool(name="outp", bufs=1) as opool,
        ):
            lhsT_sb = wpool.tile([KROWS, N], mybir.dt.bfloat16)
            rhsb_sb = wpool.tile([KROWS, TOTW], mybir.dt.bfloat16)
            # interleave weight/candidate chunks in consumption order so the
            # first slots start as early as possible (the queue is serial)
            nc.sync.dma_start(out=lhsT_sb[:, :2048], in_=lhsT_d[:, :2048])
            nc.sync.dma_start(
                out=rhsb_sb[:, TIER_OFF[0]:TIER_OFF[8]],
                in_=rhsb_d[:, TIER_OFF[0]:TIER_OFF[8]],
            )
            nc.sync.dma_start(
                out=rhsb_sb[:, TIER_OFF[8]:TIER_OFF[16]],
                in_=rhsb_d[:, TIER_OFF[8]:TIER_OFF[16]],
            )
            nc.sync.dma_start(out=lhsT_sb[:, 2048:], in_=lhsT_d[:, 2048:])
            for c in range(16, NBLK, 8):
                nc.sync.dma_start(
                    out=rhsb_sb[:, TIER_OFF[c]:TIER_OFF[c + 8]],
                    in_=rhsb_d[:, TIER_OFF[c]:TIER_OFF[c + 8]],
                )

            outacc = opool.tile([128, NBLK], mybir.dt.float32)

            for s in range(NBLK):
                w = TIER_W[s]
                ps = pspool.tile([128, w], mybir.dt.float32)
                nc.tensor.matmul(
                    ps,
                    lhsT_sb[:, s * 128:(s + 1) * 128],
                    rhsb_sb[:, TIER_OFF[s]:TIER_OFF[s + 1]],
                    start=True,
                    stop=True,
                )
                # pool reads PSUM directly (runs at the same 1x rate as a
                # pool from SBUF would, so the evacuation pass is pure waste)
                nc.vector.pool_max(outacc[:, s:s + 1], ps)
            nc.sync.dma_start(out=out_d[:, :], in_=outacc)

    nc.compile()
    return nc


def _get_program():
    global _PROGRAM
    if _PROGRAM is None:
        _PROGRAM = _build_program()
    return _PROGRAM


def build_in_maps(source, target):
    """Host prep: returns (in_maps, meta) where meta holds per-job unsort info."""
    in_maps, meta = [], []
    for b in range(B):
        L, R = _build_planes(source[b], target[b])      # rows=src planes, cand=tgt planes
        L2, R2 = _build_planes(target[b], source[b])    # rows=tgt planes, cand=src planes
        for direction in (0, 1):
            if direction == 0:
                lhsT, rhsb, order, r_cover, forced = _prep_job(source[b], target[b], L, R)
            else:
                lhsT, rhsb, order, r_cover, forced = _prep_job(target[b], source[b], L2, R2)
            in_maps.append({"lhsT": lhsT, "rhsb": rhsb})
            meta.append((b, direction, order, r_cover, forced))
    return in_maps, meta


def _exact_minsq_fp64(pts, others):
    """Exact (fp64) min squared distance from each of pts to the set others."""
    p = pts.astype(np.float64)
    o = others.astype(np.float64)
    no = (o * o).sum(1)
    out = np.empty(len(p), np.float64)
    for i0 in range(0, len(p), 2048):
        pp = p[i0:i0 + 2048]
        sq = ((pp * pp).sum(1))[:, None] + no[None, :] - 2.0 * (pp @ o.T)
        out[i0:i0 + 2048] = sq.min(1)
    return np.maximum(out, 0.0)


def kernel(source, target, weights):
    from concourse.bass_utils import run_bass_kernel_spmd

    source = np.asarray(source)
    target = np.asarray(target)
    weights = np.asarray(weights)

    in_maps, meta = build_in_maps(source, target)

    nc = _get_program()
    res = None
    last_err = None
    for attempt in range(3):
        try:
            res = run_bass_kernel_spmd(nc, in_maps, list(range(NCORES))).results
            break
        except Exception as e:  # transient device wedge: retry
            last_err = e
            import time as _time

            _time.sleep(5.0 * (attempt + 1))
    if res is None:
        raise last_err

    s_minsq = np.empty((B, N), np.float64)
    t_minsq = np.empty((B, M), np.float64)
    for j in range(NCORES):
        b, direction, order, r_cover, forced = meta[j]
        wmin_sorted = np.maximum(-res[j]["out"].T.reshape(-1).astype(np.float64), 0.0)
        # certificate: exact unless min reaches the covered-region boundary
        bad = forced | (np.sqrt(wmin_sorted) >= CERT_MARGIN * r_cover) | (wmin_sorted < SMALL_SQ_THRESH)
        pts = source[b] if direction == 0 else target[b]
        others = target[b] if direction == 0 else source[b]
        bad_rows = order[np.flatnonzero(bad)]
        full = np.empty(len(pts), np.float64)
        full[order] = wmin_sorted
        if len(bad_rows):
            full[bad_rows] = _exact_minsq_fp64(pts[bad_rows], others)
        if direction == 0:
            s_minsq[b] = full
        else:
            t_minsq[b] = full

    fwd = float((np.sqrt(s_minsq + EPS) * weights.astype(np.float64)).mean())
    bwd = float(np.sqrt(t_minsq + EPS).mean())
    return np.float32(fwd + bwd)
